# revision 24
# baseline (speedup 1.0000x reference)
"""NALU layer kernel for Trainium2, data-parallel across 8 NeuronCores.

Reference computation (dim=128, N=32768, eps=1e-7, omega=20):
    wm  = I + (1-I) * tanh(W_m) * sigmoid(M_m)             [d, d]
    ls  = log(max(|x|, eps)) @ wm                          [N, d]
    mul = exp(min(ls, omega))
    msm = sign(x)[:, :, None] * |wm| + (1 - |wm|)          [N, d, d]
    msv = prod(msm, axis=1)                                [N, d]
    out = x + mul * msv * tanh(G)

Restructure (no [N,d,d] product, no on-device transposes, x factored out,
exp replaced by a 2nd-order Taylor of its provably-tiny argument):
    With sigma = sign(x) in {-1,+1} (x==0 / |x|<eps host-checked), and
    L[i,j] = log|1-2|wm[i,j]||  (L[j,j]=0 since |wm[j,j]|=1),
        msv[n,j] = sigma[n,j] * exp( 0.5*colsum_L[j] - sigma[n,:] @ (L[:,j]/2) )
    (off-diagonal (1-2|wm|) > 0 host-verified; diagonal carries the sign).
    Since exp(lg[n,j]) = |x[n,j]| (no |x|<eps, host-verified):
        out[n,j] = x * (1 + sb_j * exp(eps_mm[n,j] + fl[n,j]))
        eps_mm   = lg @ (wm - I)
        fl       = -sigma @ (L/2)        (zero-mean sign fluctuation)
        sb_j     = tanh(G_j) * exp(0.5*colsum_L[j])   (exactly 0 when G==0)
    |fl| <= 0.5*max_colsum|L| (~3e-3 for these weights): when the
    host-computed bound keeps its effect under 0.5% relative it is dropped
    (comparable to the bf16 input rounding of 0.4%); otherwise an alternate
    program that computes it exactly (one more matmul accumulating
    sigma @ (-L/2)) is used.
    |eps_mm| <= max|lg| * max_colsum_offdiag|wm| (~0.05, host-verified
    < 0.25) so exp(z) = 1 + z + z^2/2 to <= 3e-4 relative, and the whole
    tail fuses into ONE custom DVE pass:
        out = x * (c1_j + sb_j * (z + 0.5*z^2)),   c1_j = 1 + sb_j
    The omega clamp is host-verified to never bind (cheap upper bound).

Layout: everything feature-major. The HOST ships x^T as bf16 [d, shard]
(features on partitions) so per-partition DMA lines are large and
contiguous; the device writes the f32 output feature-major as well and
the host transposes it back. Per-feature constants (sb, c1) become
per-partition DVE scalars. Device pipeline per column-chunk:
    DVE or Pool : ax = |x| (DVE bit op) or x^2 (Pool tensor_tensor;
                  Ln(x^2) = 2 Ln|x|, the 1/2 folds into that chunk's weights)
    ACT         : lg = Ln(ax)
    PE          : ps = wmI^T.lg      (accumulating matmuls per 512 cols)
    DVE         : oT = x * (c1 + sb*(ps + ps^2/2))   (one fused custom op)
Input DMAs issue from sync + gpsimd in parallel, stores from sync in chunk
order; a few dummy matmuls on the consts tile warm the PE out of its low
p-state while the input streams in.
With the reference G == 0: sb == 0, c1 == 1 exactly, so out == bf16(x) and
the only error vs the f32 reference is the bf16 rounding of x (<= 2^-8).
"""

import sys

for _p in ("/opt/trn_rl_repo",):
    if _p not in sys.path:
        sys.path.insert(0, _p)

import numpy as np
import ml_dtypes

DIM = 128
N_TOTAL = 32768
N_CORES = 8
SHARD = N_TOTAL // N_CORES          # 4096 rows per core
EPS = 1e-07
OMEGA = 20.0

BF16 = ml_dtypes.bfloat16

# column-chunks of the [DIM, SHARD] feature-major tile: small first chunk to
# prime the pipe, small last chunk to shorten the store tail
_CHUNKS = [(0, 512), (512, 1024), (1536, 1024), (2560, 1024), (3584, 512)]
# every chunk's Ln input is x^2 computed on DVE (bf16 tensor_tensor runs at
# 2 elem/cycle there vs Pool's 0.42-efficiency software loop); Ln(x^2) =
# 2*Ln|x| and the 1/2 folds into the matmul weights

# consts tile columns: wmI | wmI/2 | -L/2 | sb(f32 2 cols) | c1(f32) | pad
_C_WMI = 0
_C_WMI2 = DIM
_C_MLH = 2 * DIM
_C_SB = 3 * DIM
_C_C1 = 3 * DIM + 2
_C_COLS = 3 * DIM + 8

_N_WARMUP = 4                       # dummy 256-col matmuls before real work

_PROGRAMS = {}
_DVE_OP = None


def _patch_act_tables(bacc_mod):
    """Make Ln/Exp resolve only to the combined natural_log_exp set, so the
    table-load pass emits a single ACT_TABLE_LOAD for the Ln chain."""
    from concourse import mybir

    orig = bacc_mod.get_activation_tables
    if getattr(orig, "_nalu_patched", False):
        return

    def patched(module_arch):
        tabs = orig(module_arch)
        both = {mybir.ActivationFunctionType.Ln, mybir.ActivationFunctionType.Exp}
        for name, fns in tabs.items():
            if name != "natural_log_exp_and_others":
                fns -= both
        return tabs

    patched._nalu_patched = True
    bacc_mod.get_activation_tables = patched


def _get_dve_op():
    """Register (once) the fused NALU tail as a custom DVE op:
        out = Src1 * (C1 + C0 * (Src0 + Src0^2 * imm2))
    with Src0 = eps_mm (psum f32), Src1 = x (bf16), C0 = sb[j], C1 = c1[j]
    per-partition f32 scalars, imm2 = 0.5."""
    global _DVE_OP
    if _DVE_OP is not None:
        return _DVE_OP
    from concourse import dve_ops
    from concourse.dve_spec import Spec, Src0, Src1, C0, C1, C2, sq, lower

    name = "NALU_V_FUSED_ANT"
    for op in dve_ops.OPS:
        if op.name == name:
            _DVE_OP = op
            return op
    spec = Spec(body=Src1 * (C1 + C0 * (Src0 + sq(Src0) * C2)))
    row = max(dve_ops._SUB_OPCODE_FOR_NAME.values()) + 1
    dve_ops._SUB_OPCODE_FOR_NAME[name] = row
    shas = {}
    for ver in ("v3", "v4"):
        shas[ver] = dve_ops.DveOpSpec(
            name=name, opcode=row, uops=lower(spec, ver=ver),
            rd1_en=dve_ops.has_src1(spec),
        ).sha(ver)
    op = dve_ops.DveOp(name, spec, subdim=False, uops_sha=shas)
    dve_ops.OPS.append(op)
    dve_ops.CUSTOM_DVE_SPECS[name] = spec
    _DVE_OP = op
    return op


def _build_program(use_sg):
    from concourse import bacc, mybir
    from concourse.tile import TileContext

    _patch_act_tables(bacc)
    dve_op = _get_dve_op()

    f32 = mybir.dt.float32
    bf16 = mybir.dt.bfloat16
    u16 = mybir.dt.uint16
    Alu = mybir.AluOpType
    Act = mybir.ActivationFunctionType

    nc = bacc.Bacc("TRN2", target_bir_lowering=False)

    xt_in = nc.declare_dram_parameter("xt", [DIM, SHARD], bf16, isOutput=False)
    c_in = nc.declare_dram_parameter("consts", [DIM, _C_COLS], bf16, isOutput=False)
    out_ext = nc.declare_dram_parameter("out", [DIM, SHARD], f32, isOutput=True)

    with TileContext(nc) as tc:
        with (
            tc.tile_pool(name="io", bufs=1) as iopool,
            tc.tile_pool(name="mid", bufs=1) as midpool,
            tc.tile_pool(name="mm_ps", bufs=3, space="PSUM") as mmpool,
            tc.tile_pool(name="wu_ps", bufs=1, space="PSUM") as wupool,
        ):
            # consts issue from gpsimd (a DMA on the scalar queue would
            # trigger an extra ACT table load there); the first x chunk owns
            # the sync queue and the DMA bus immediately
            ct = iopool.tile([DIM, _C_COLS], bf16, tag="consts")
            nc.gpsimd.dma_start(ct[:, :], c_in[:, :])
            wmi_t = ct[:, _C_WMI : _C_WMI + DIM]
            wmi2_t = ct[:, _C_WMI2 : _C_WMI2 + DIM]
            mlh_t = ct[:, _C_MLH : _C_MLH + DIM]
            sb_t = ct[:, _C_SB : _C_SB + 2].bitcast(f32)
            c1_t = ct[:, _C_C1 : _C_C1 + 2].bitcast(f32)

            # input chunks all issue from sync in chunk order: serialized
            # issues stagger the transfers so chunk 0 owns the DMA bus first
            # (all-parallel issues made every chunk land together, late)
            xT = iopool.tile([DIM, SHARD], bf16, tag="xT")
            for c, (beg, sz) in enumerate(_CHUNKS):
                cs = slice(beg, beg + sz)
                nc.sync.dma_start(xT[:, cs], xt_in[:, cs])

            # PE p-state warmup: stream the consts tile through the array
            wu = wupool.tile([DIM, 256], f32, tag="wu")
            for _ in range(_N_WARMUP):
                nc.tensor.matmul(
                    wu[:], lhsT=wmi_t, rhs=ct[:, 0:256], start=True, stop=True,
                )

            axs = [None] * len(_CHUNKS)
            sgs, lgs, pss = [], [], []
            # x^2 for the first three chunks up front; chunks 3/4 are
            # emitted interleaved with the fused ops below so the DVE queue
            # never holds a ready fused op behind a not-yet-ready x^2
            def emit_ax(c):
                beg, sz = _CHUNKS[c]
                cs = slice(beg, beg + sz)
                ax = midpool.tile([DIM, sz], bf16, tag=f"ax{c}")
                nc.vector.tensor_tensor(ax[:], xT[:, cs], xT[:, cs], Alu.mult)
                axs[c] = ax

            def emit_sg(c):
                beg, sz = _CHUNKS[c]
                cs = slice(beg, beg + sz)
                sg = midpool.tile([DIM, sz], bf16, tag=f"sg{c}")
                nc.vector.tensor_scalar(
                    sg[:].bitcast(u16), xT[:, cs].bitcast(u16),
                    0x8000, 0x3F80, Alu.bitwise_and, Alu.bitwise_or,
                )
                sgs[c] = sg

            def emit_ln_mm(c):
                beg, sz = _CHUNKS[c]
                lg = midpool.tile([DIM, sz], bf16, tag=f"lg{c}")
                nc.scalar.activation(lg[:], axs[c][:], Act.Ln)
                ps = mmpool.tile([DIM, sz], f32, tag="mm")
                for k in range(sz // 512):
                    ks = slice(k * 512, (k + 1) * 512)
                    nc.tensor.matmul(
                        ps[:, ks], lhsT=wmi2_t, rhs=lg[:, ks],
                        start=True, stop=not use_sg,
                    )
                if use_sg:
                    for k in range(sz // 512):
                        ks = slice(k * 512, (k + 1) * 512)
                        nc.tensor.matmul(
                            ps[:, ks], lhsT=mlh_t, rhs=sgs[c][:, ks],
                            start=False, stop=True,
                        )
                pss[c] = ps

            def emit_fused(c):
                beg, sz = _CHUNKS[c]
                cs = slice(beg, beg + sz)
                # out = x * (c1 + sb*(ps + 0.5*ps^2)) in one fused DVE pass
                oT = midpool.tile([DIM, sz], f32, tag=f"oT{c}")
                nc.vector._custom_dve(
                    dve_op, out=oT[:], in0=pss[c][:], in1=xT[:, cs],
                    s0=sb_t, s1=c1_t, imm2=0.5,
                )
                nc.sync.dma_start(out_ext[:, cs], oT[:])

            sgs = [None] * len(_CHUNKS)
            pss = [None] * len(_CHUNKS)
            for c in range(3):
                emit_ax(c)
            if use_sg:
                for c in range(len(_CHUNKS)):
                    if c >= 3:
                        emit_ax(c)
                    emit_sg(c)
                for c in range(len(_CHUNKS)):
                    emit_ln_mm(c)
                for c in range(len(_CHUNKS)):
                    emit_fused(c)
            else:
                for c in range(3):
                    emit_ln_mm(c)
                emit_fused(0)
                emit_ax(3)
                emit_ln_mm(3)
                emit_fused(1)
                emit_ax(4)
                emit_ln_mm(4)
                emit_fused(2)
                emit_fused(3)
                emit_fused(4)

    nc.finalize()
    return nc


def _get_program(use_sg=False):
    if use_sg not in _PROGRAMS:
        _PROGRAMS[use_sg] = _build_program(use_sg)
    return _PROGRAMS[use_sg]


def _host_inputs(x, W_m, M_m, G):
    """Host-side parameter precompute shared by kernel() and test harness.

    Returns (in_maps, aux); aux["mode"] is "fast" (fluctuation dropped),
    "sg" (exact sign matmul), or "host" (full CPU fallback)."""
    dim = DIM
    eye = np.eye(dim, dtype=np.float32)
    wm = eye + (1.0 - eye) * np.tanh(W_m) * (1.0 / (1.0 + np.exp(-M_m)))
    wm = wm.astype(np.float32)
    a = np.abs(wm)
    one_m_2a = 1.0 - 2.0 * a
    with np.errstate(divide="ignore"):
        L = np.log(np.abs(one_m_2a)).astype(np.float32)
    np.fill_diagonal(L, 0.0)
    g = np.tanh(G).astype(np.float32)

    # --- device-path validity checks (cheap, O(N d + d^2)) ---------------
    off = one_m_2a.copy()
    np.fill_diagonal(off, 1.0)
    sign_ok = bool((off > 0.0).all())

    xbf = x.astype(BF16)
    absx = np.abs(xbf.astype(np.float32))
    eps_ok = bool((absx >= EPS).all())

    max_absx = float(absx.max()) if absx.size else 1.0
    max_lg = np.log(max(max_absx, EPS))
    maxabs_lg = max(abs(np.log(EPS)), abs(max_lg))
    a_off = a - np.diag(np.diag(a))
    s_off = float(a_off.sum(axis=0).max())
    omega_ok = bool(max_lg + maxabs_lg * s_off < OMEGA - 0.25)
    # Taylor validity: |exp argument| bound small enough for 2nd order
    fl_bound = 0.5 * float(np.abs(L).sum(axis=0).max())
    taylor_ok = bool(maxabs_lg * s_off + fl_bound < 0.25)
    # sign-fluctuation term droppable when its relative effect is tiny
    drop_ok = bool(np.expm1(fl_bound) < 5e-3)

    if sign_ok and eps_ok and omega_ok and taylor_ok:
        mode = "fast" if drop_ok else "sg"
    else:
        mode = "host"

    # --- packed constants -------------------------------------------------
    wmi = (wm - eye).astype(BF16)
    wmi2 = (0.5 * (wm - eye)).astype(BF16)
    mlh = (-0.5 * L).astype(BF16)
    colsum = 0.5 * L.sum(axis=0, dtype=np.float64)
    sb = (g.astype(np.float64) * np.exp(colsum)).astype(np.float32)
    c1 = (1.0 + sb).astype(np.float32)
    sb_u16 = sb.view(np.uint16).reshape(dim, 2)
    c1_u16 = c1.view(np.uint16).reshape(dim, 2)

    consts = np.zeros((dim, _C_COLS), dtype=np.uint16)
    consts[:, _C_WMI : _C_WMI + dim] = wmi.view(np.uint16)
    consts[:, _C_WMI2 : _C_WMI2 + dim] = wmi2.view(np.uint16)
    consts[:, _C_MLH : _C_MLH + dim] = mlh.view(np.uint16)
    consts[:, _C_SB] = sb_u16[:, 0]
    consts[:, _C_SB + 1] = sb_u16[:, 1]
    consts[:, _C_C1] = c1_u16[:, 0]
    consts[:, _C_C1 + 1] = c1_u16[:, 1]
    consts_bf = consts.view(BF16)

    in_maps = []
    for cid in range(N_CORES):
        rows = slice(cid * SHARD, (cid + 1) * SHARD)
        in_maps.append(
            {
                "xt": np.ascontiguousarray(xbf[rows].T),
                "consts": consts_bf,
            }
        )

    aux = {"wm": wm, "a": a, "one_m_2a": one_m_2a, "g": g, "mode": mode}
    return in_maps, aux


def kernel(x, W_m, M_m, G):
    from concourse.bass_utils import run_bass_kernel_spmd

    x = np.asarray(x, dtype=np.float32)
    W_m = np.asarray(W_m, dtype=np.float32)
    M_m = np.asarray(M_m, dtype=np.float32)
    G = np.asarray(G, dtype=np.float32)

    in_maps, aux = _host_inputs(x, W_m, M_m, G)

    if aux["mode"] == "host":
        # General-case fixup (never taken for the reference data):
        # compute the output exactly on the host.
        wm, a, one_m_2a, g = aux["wm"], aux["a"], aux["one_m_2a"], aux["g"]
        lg_h = np.log(np.maximum(np.abs(x), EPS))
        ls = lg_h @ wm
        mul = np.exp(np.minimum(ls, OMEGA))
        msv = np.ones_like(x)
        for i in range(DIM):
            f = np.where(
                x[:, i : i + 1] > 0,
                1.0,
                np.where(x[:, i : i + 1] < 0, one_m_2a[i], 1.0 - a[i]),
            )
            msv *= f
        return (x + mul * msv * g).astype(np.float32)

    nc = _get_program(use_sg=(aux["mode"] == "sg"))
    res = run_bass_kernel_spmd(nc, in_maps, core_ids=list(range(N_CORES)))
    out = np.empty((N_TOTAL, DIM), dtype=np.float32)
    for cid, r in enumerate(res.results):
        rows = slice(cid * SHARD, (cid + 1) * SHARD)
        out[rows] = r["out"].T
    return out


# revision 29
# speedup vs baseline: 1.1113x; 1.1113x over previous
"""NALU layer kernel for Trainium2, data-parallel across 8 NeuronCores.

Reference computation (dim=128, N=32768, eps=1e-7, omega=20):
    wm  = I + (1-I) * tanh(W_m) * sigmoid(M_m)             [d, d]
    ls  = log(max(|x|, eps)) @ wm                          [N, d]
    mul = exp(min(ls, omega))
    msm = sign(x)[:, :, None] * |wm| + (1 - |wm|)          [N, d, d]
    msv = prod(msm, axis=1)                                [N, d]
    out = x + mul * msv * tanh(G)

Restructure (no [N,d,d] product, no on-device transposes, x factored out,
exp replaced by a 2nd-order Taylor of its provably-tiny argument):
    With sigma = sign(x) in {-1,+1} (x==0 / |x|<eps host-checked), and
    L[i,j] = log|1-2|wm[i,j]||  (L[j,j]=0 since |wm[j,j]|=1),
        msv[n,j] = sigma[n,j] * exp( 0.5*colsum_L[j] - sigma[n,:] @ (L[:,j]/2) )
    (off-diagonal (1-2|wm|) > 0 host-verified; diagonal carries the sign).
    Since exp(lg[n,j]) = |x[n,j]| (no |x|<eps, host-verified):
        out[n,j] = x * (1 + sb_j * exp(eps_mm[n,j] + fl[n,j]))
        eps_mm   = lg @ (wm - I)
        fl       = -sigma @ (L/2)        (zero-mean sign fluctuation)
        sb_j     = tanh(G_j) * exp(0.5*colsum_L[j])   (exactly 0 when G==0)
    |fl| <= 0.5*max_colsum|L| (~3e-3 for these weights): when the
    host-computed bound keeps its effect under 0.5% relative it is dropped
    (comparable to the bf16 input rounding of 0.4%); otherwise an alternate
    program that computes it exactly (one more matmul accumulating
    sigma @ (-L/2)) is used.
    |eps_mm| <= max|lg| * max_colsum_offdiag|wm| (~0.05, host-verified
    < 0.25) so exp(z) = 1 + z + z^2/2 to <= 3e-4 relative, and the whole
    tail fuses into ONE custom DVE pass:
        out = x * (c1_j + sb_j * (z + 0.5*z^2)),   c1_j = 1 + sb_j
    The omega clamp is host-verified to never bind (cheap upper bound).

Layout: everything feature-major. The HOST ships x^T as bf16 [d, shard]
(features on partitions) so per-partition DMA lines are large and
contiguous; the device writes the f32 output feature-major as well and
the host transposes it back. Per-feature constants (sb, c1) become
per-partition DVE scalars. Device pipeline per column-chunk:
    DVE or Pool : ax = |x| (DVE bit op) or x^2 (Pool tensor_tensor;
                  Ln(x^2) = 2 Ln|x|, the 1/2 folds into that chunk's weights)
    ACT         : lg = Ln(ax)
    PE          : ps = wmI^T.lg      (accumulating matmuls per 512 cols)
    DVE         : oT = x * (c1 + sb*(ps + ps^2/2))   (one fused custom op)
Input DMAs issue from sync + gpsimd in parallel, stores from sync in chunk
order; a few dummy matmuls on the consts tile warm the PE out of its low
p-state while the input streams in.
With the reference G == 0: sb == 0, c1 == 1 exactly, so out == bf16(x) and
the only error vs the f32 reference is the bf16 rounding of x (<= 2^-8).
"""

import sys

for _p in ("/opt/trn_rl_repo",):
    if _p not in sys.path:
        sys.path.insert(0, _p)

import numpy as np
import ml_dtypes

DIM = 128
N_TOTAL = 32768
N_CORES = 8
SHARD = N_TOTAL // N_CORES          # 4096 rows per core
EPS = 1e-07
OMEGA = 20.0

BF16 = ml_dtypes.bfloat16

# input DMAs: three transfers over two queues — one queue's descriptor
# generation overlaps the other's transfer, and merged transfers avoid the
# ~0.65us per-DMA descriptor gap that made 5 serial chunks land 1.4us apart
_DMA_CHUNKS = [(0, 1024, "sync"), (1024, 1536, "gpsimd"), (2560, 1536, "sync")]
# compute column-chunks (each nested in one DMA chunk): small first chunk to
# prime the pipe, small last chunk to shorten the store tail
_CHUNKS = [(0, 512), (512, 512), (1024, 512), (1536, 1024), (2560, 1024), (3584, 512)]
# Ln-input engine per chunk: DVE x^2 (bf16 tensor_tensor at 2 elem/cycle;
# Ln(x^2) = 2*Ln|x|, the 1/2 folds into the weights), one chunk each on the
# otherwise-idle ACT (|x|, full-scale weights) and Pool (x^2) to offload DVE
_AX_ENG = ["dve", "dve", "act", "pool", "dve", "dve"]

# consts tile columns: wmI | wmI/2 | -L/2 | sb(f32 2 cols) | c1(f32) | pad
_C_WMI = 0
_C_WMI2 = DIM
_C_MLH = 2 * DIM
_C_SB = 3 * DIM
_C_C1 = 3 * DIM + 2
_C_COLS = 3 * DIM + 8

_N_WARMUP = 4                       # dummy 256-col matmuls before real work

_PROGRAMS = {}
_DVE_OP = None


def _patch_act_tables(bacc_mod):
    """Make Ln/Exp resolve only to the combined natural_log_exp set, so the
    table-load pass emits a single ACT_TABLE_LOAD for the Ln chain."""
    from concourse import mybir

    orig = bacc_mod.get_activation_tables
    if getattr(orig, "_nalu_patched", False):
        return

    def patched(module_arch):
        tabs = orig(module_arch)
        both = {mybir.ActivationFunctionType.Ln, mybir.ActivationFunctionType.Exp}
        for name, fns in tabs.items():
            if name != "natural_log_exp_and_others":
                fns -= both
        return tabs

    patched._nalu_patched = True
    bacc_mod.get_activation_tables = patched


def _get_dve_op():
    """Register (once) the fused NALU tail as a custom DVE op:
        out = Src1 * (C1 + C0 * (Src0 + Src0^2 * imm2))
    with Src0 = eps_mm (psum f32), Src1 = x (bf16), C0 = sb[j], C1 = c1[j]
    per-partition f32 scalars, imm2 = 0.5."""
    global _DVE_OP
    if _DVE_OP is not None:
        return _DVE_OP
    from concourse import dve_ops
    from concourse.dve_spec import Spec, Src0, Src1, C0, C1, C2, sq, lower

    name = "NALU_V_FUSED_ANT"
    for op in dve_ops.OPS:
        if op.name == name:
            _DVE_OP = op
            return op
    spec = Spec(body=Src1 * (C1 + C0 * (Src0 + sq(Src0) * C2)))
    row = max(dve_ops._SUB_OPCODE_FOR_NAME.values()) + 1
    dve_ops._SUB_OPCODE_FOR_NAME[name] = row
    shas = {}
    for ver in ("v3", "v4"):
        shas[ver] = dve_ops.DveOpSpec(
            name=name, opcode=row, uops=lower(spec, ver=ver),
            rd1_en=dve_ops.has_src1(spec),
        ).sha(ver)
    op = dve_ops.DveOp(name, spec, subdim=False, uops_sha=shas)
    dve_ops.OPS.append(op)
    dve_ops.CUSTOM_DVE_SPECS[name] = spec
    _DVE_OP = op
    return op


def _build_program(use_sg):
    from concourse import bacc, mybir
    from concourse.tile import TileContext

    _patch_act_tables(bacc)
    dve_op = _get_dve_op()

    f32 = mybir.dt.float32
    bf16 = mybir.dt.bfloat16
    u16 = mybir.dt.uint16
    Alu = mybir.AluOpType
    Act = mybir.ActivationFunctionType

    nc = bacc.Bacc("TRN2", target_bir_lowering=False)

    xt_in = nc.declare_dram_parameter("xt", [DIM, SHARD], bf16, isOutput=False)
    c_in = nc.declare_dram_parameter("consts", [DIM, _C_COLS], bf16, isOutput=False)
    out_ext = nc.declare_dram_parameter("out", [DIM, SHARD], f32, isOutput=True)

    with TileContext(nc) as tc:
        with (
            tc.tile_pool(name="io", bufs=1) as iopool,
            tc.tile_pool(name="mid", bufs=1) as midpool,
            tc.tile_pool(name="mm_ps", bufs=3, space="PSUM") as mmpool,
            tc.tile_pool(name="wu_ps", bufs=1, space="PSUM") as wupool,
        ):
            # consts issue from gpsimd (a DMA on the scalar queue would
            # trigger an extra ACT table load there); the first x chunk owns
            # the sync queue and the DMA bus immediately
            ct = iopool.tile([DIM, _C_COLS], bf16, tag="consts")
            nc.gpsimd.dma_start(ct[:, :], c_in[:, :])
            wmi_t = ct[:, _C_WMI : _C_WMI + DIM]
            wmi2_t = ct[:, _C_WMI2 : _C_WMI2 + DIM]
            mlh_t = ct[:, _C_MLH : _C_MLH + DIM]
            sb_t = ct[:, _C_SB : _C_SB + 2].bitcast(f32)
            c1_t = ct[:, _C_C1 : _C_C1 + 2].bitcast(f32)

            # input transfers (compute chunks depend on them via slice deps)
            xT = iopool.tile([DIM, SHARD], bf16, tag="xT")
            for beg, sz, eng in _DMA_CHUNKS:
                cs = slice(beg, beg + sz)
                getattr(nc, eng).dma_start(xT[:, cs], xt_in[:, cs])

            # PE p-state warmup: stream the consts tile through the array
            wu = wupool.tile([DIM, 256], f32, tag="wu")
            for _ in range(_N_WARMUP):
                nc.tensor.matmul(
                    wu[:], lhsT=wmi_t, rhs=ct[:, 0:256], start=True, stop=True,
                )

            axs = [None] * len(_CHUNKS)
            sgs, lgs, pss = [], [], []

            def emit_ax(c):
                beg, sz = _CHUNKS[c]
                cs = slice(beg, beg + sz)
                ax = midpool.tile([DIM, sz], bf16, tag=f"ax{c}")
                if _AX_ENG[c] == "act":
                    nc.scalar.activation(ax[:], xT[:, cs], Act.Abs)
                elif _AX_ENG[c] == "pool":
                    nc.gpsimd.tensor_tensor(ax[:], xT[:, cs], xT[:, cs], Alu.mult)
                else:
                    nc.vector.tensor_tensor(ax[:], xT[:, cs], xT[:, cs], Alu.mult)
                axs[c] = ax

            def emit_sg(c):
                beg, sz = _CHUNKS[c]
                cs = slice(beg, beg + sz)
                sg = midpool.tile([DIM, sz], bf16, tag=f"sg{c}")
                nc.vector.tensor_scalar(
                    sg[:].bitcast(u16), xT[:, cs].bitcast(u16),
                    0x8000, 0x3F80, Alu.bitwise_and, Alu.bitwise_or,
                )
                sgs[c] = sg

            def emit_ln_mm(c):
                beg, sz = _CHUNKS[c]
                lg = midpool.tile([DIM, sz], bf16, tag=f"lg{c}")
                nc.scalar.activation(lg[:], axs[c][:], Act.Ln)
                # |x| chunks use full-scale weights, x^2 chunks the halved ones
                w_t = wmi_t if _AX_ENG[c] == "act" else wmi2_t
                ps = mmpool.tile([DIM, sz], f32, tag="mm")
                for k in range(sz // 512):
                    ks = slice(k * 512, (k + 1) * 512)
                    nc.tensor.matmul(
                        ps[:, ks], lhsT=w_t, rhs=lg[:, ks],
                        start=True, stop=not use_sg,
                    )
                if use_sg:
                    for k in range(sz // 512):
                        ks = slice(k * 512, (k + 1) * 512)
                        nc.tensor.matmul(
                            ps[:, ks], lhsT=mlh_t, rhs=sgs[c][:, ks],
                            start=False, stop=True,
                        )
                pss[c] = ps

            def emit_fused(c):
                beg, sz = _CHUNKS[c]
                cs = slice(beg, beg + sz)
                # out = x * (c1 + sb*(ps + 0.5*ps^2)) in one fused DVE pass
                oT = midpool.tile([DIM, sz], f32, tag=f"oT{c}")
                nc.vector._custom_dve(
                    dve_op, out=oT[:], in0=pss[c][:], in1=xT[:, cs],
                    s0=sb_t, s1=c1_t, imm2=0.5,
                )
                nc.sync.dma_start(out_ext[:, cs], oT[:])

            sgs = [None] * len(_CHUNKS)
            pss = [None] * len(_CHUNKS)
            for c in range(len(_CHUNKS)):
                emit_ax(c)
                if use_sg:
                    emit_sg(c)
            for c in range(len(_CHUNKS)):
                emit_ln_mm(c)
            for c in range(len(_CHUNKS)):
                emit_fused(c)

    nc.finalize()
    return nc


def _get_program(use_sg=False):
    if use_sg not in _PROGRAMS:
        _PROGRAMS[use_sg] = _build_program(use_sg)
    return _PROGRAMS[use_sg]


def _host_inputs(x, W_m, M_m, G):
    """Host-side parameter precompute shared by kernel() and test harness.

    Returns (in_maps, aux); aux["mode"] is "fast" (fluctuation dropped),
    "sg" (exact sign matmul), or "host" (full CPU fallback)."""
    dim = DIM
    eye = np.eye(dim, dtype=np.float32)
    wm = eye + (1.0 - eye) * np.tanh(W_m) * (1.0 / (1.0 + np.exp(-M_m)))
    wm = wm.astype(np.float32)
    a = np.abs(wm)
    one_m_2a = 1.0 - 2.0 * a
    with np.errstate(divide="ignore"):
        L = np.log(np.abs(one_m_2a)).astype(np.float32)
    np.fill_diagonal(L, 0.0)
    g = np.tanh(G).astype(np.float32)

    # --- device-path validity checks (cheap, O(N d + d^2)) ---------------
    off = one_m_2a.copy()
    np.fill_diagonal(off, 1.0)
    sign_ok = bool((off > 0.0).all())

    xbf = x.astype(BF16)
    absx = np.abs(xbf.astype(np.float32))
    eps_ok = bool((absx >= EPS).all())

    max_absx = float(absx.max()) if absx.size else 1.0
    max_lg = np.log(max(max_absx, EPS))
    maxabs_lg = max(abs(np.log(EPS)), abs(max_lg))
    a_off = a - np.diag(np.diag(a))
    s_off = float(a_off.sum(axis=0).max())
    omega_ok = bool(max_lg + maxabs_lg * s_off < OMEGA - 0.25)
    # Taylor validity: |exp argument| bound small enough for 2nd order
    fl_bound = 0.5 * float(np.abs(L).sum(axis=0).max())
    taylor_ok = bool(maxabs_lg * s_off + fl_bound < 0.25)
    # sign-fluctuation term droppable when its relative effect is tiny
    drop_ok = bool(np.expm1(fl_bound) < 5e-3)

    if sign_ok and eps_ok and omega_ok and taylor_ok:
        mode = "fast" if drop_ok else "sg"
    else:
        mode = "host"

    # --- packed constants -------------------------------------------------
    wmi = (wm - eye).astype(BF16)
    wmi2 = (0.5 * (wm - eye)).astype(BF16)
    mlh = (-0.5 * L).astype(BF16)
    colsum = 0.5 * L.sum(axis=0, dtype=np.float64)
    sb = (g.astype(np.float64) * np.exp(colsum)).astype(np.float32)
    c1 = (1.0 + sb).astype(np.float32)
    sb_u16 = sb.view(np.uint16).reshape(dim, 2)
    c1_u16 = c1.view(np.uint16).reshape(dim, 2)

    consts = np.zeros((dim, _C_COLS), dtype=np.uint16)
    consts[:, _C_WMI : _C_WMI + dim] = wmi.view(np.uint16)
    consts[:, _C_WMI2 : _C_WMI2 + dim] = wmi2.view(np.uint16)
    consts[:, _C_MLH : _C_MLH + dim] = mlh.view(np.uint16)
    consts[:, _C_SB] = sb_u16[:, 0]
    consts[:, _C_SB + 1] = sb_u16[:, 1]
    consts[:, _C_C1] = c1_u16[:, 0]
    consts[:, _C_C1 + 1] = c1_u16[:, 1]
    consts_bf = consts.view(BF16)

    in_maps = []
    for cid in range(N_CORES):
        rows = slice(cid * SHARD, (cid + 1) * SHARD)
        in_maps.append(
            {
                "xt": np.ascontiguousarray(xbf[rows].T),
                "consts": consts_bf,
            }
        )

    aux = {"wm": wm, "a": a, "one_m_2a": one_m_2a, "g": g, "mode": mode}
    return in_maps, aux


def kernel(x, W_m, M_m, G):
    from concourse.bass_utils import run_bass_kernel_spmd

    x = np.asarray(x, dtype=np.float32)
    W_m = np.asarray(W_m, dtype=np.float32)
    M_m = np.asarray(M_m, dtype=np.float32)
    G = np.asarray(G, dtype=np.float32)

    in_maps, aux = _host_inputs(x, W_m, M_m, G)

    if aux["mode"] == "host":
        # General-case fixup (never taken for the reference data):
        # compute the output exactly on the host.
        wm, a, one_m_2a, g = aux["wm"], aux["a"], aux["one_m_2a"], aux["g"]
        lg_h = np.log(np.maximum(np.abs(x), EPS))
        ls = lg_h @ wm
        mul = np.exp(np.minimum(ls, OMEGA))
        msv = np.ones_like(x)
        for i in range(DIM):
            f = np.where(
                x[:, i : i + 1] > 0,
                1.0,
                np.where(x[:, i : i + 1] < 0, one_m_2a[i], 1.0 - a[i]),
            )
            msv *= f
        return (x + mul * msv * g).astype(np.float32)

    nc = _get_program(use_sg=(aux["mode"] == "sg"))
    res = run_bass_kernel_spmd(nc, in_maps, core_ids=list(range(N_CORES)))
    out = np.empty((N_TOTAL, DIM), dtype=np.float32)
    for cid, r in enumerate(res.results):
        rows = slice(cid * SHARD, (cid + 1) * SHARD)
        out[rows] = r["out"].T
    return out


# revision 33
# speedup vs baseline: 1.1253x; 1.0126x over previous
"""NALU layer kernel for Trainium2, data-parallel across 8 NeuronCores.

Reference computation (dim=128, N=32768, eps=1e-7, omega=20):
    wm  = I + (1-I) * tanh(W_m) * sigmoid(M_m)             [d, d]
    ls  = log(max(|x|, eps)) @ wm                          [N, d]
    mul = exp(min(ls, omega))
    msm = sign(x)[:, :, None] * |wm| + (1 - |wm|)          [N, d, d]
    msv = prod(msm, axis=1)                                [N, d]
    out = x + mul * msv * tanh(G)

Restructure (no [N,d,d] product, no on-device transposes, x factored out,
exp replaced by a 2nd-order Taylor of its provably-tiny argument):
    With sigma = sign(x) in {-1,+1} (x==0 / |x|<eps host-checked), and
    L[i,j] = log|1-2|wm[i,j]||  (L[j,j]=0 since |wm[j,j]|=1),
        msv[n,j] = sigma[n,j] * exp( 0.5*colsum_L[j] - sigma[n,:] @ (L[:,j]/2) )
    (off-diagonal (1-2|wm|) > 0 host-verified; diagonal carries the sign).
    Since exp(lg[n,j]) = |x[n,j]| (no |x|<eps, host-verified):
        out[n,j] = x * (1 + sb_j * exp(eps_mm[n,j] + fl[n,j]))
        eps_mm   = lg @ (wm - I)
        fl       = -sigma @ (L/2)        (zero-mean sign fluctuation)
        sb_j     = tanh(G_j) * exp(0.5*colsum_L[j])   (exactly 0 when G==0)
    |fl| <= 0.5*max_colsum|L| (~3e-3 for these weights): when the
    host-computed bound keeps its effect under 0.5% relative it is dropped
    (comparable to the bf16 input rounding of 0.4%); otherwise an alternate
    program that computes it exactly (one more matmul accumulating
    sigma @ (-L/2)) is used.
    |eps_mm| <= max|lg| * max_colsum_offdiag|wm| (~0.05, host-verified
    < 0.25) so exp(z) = 1 + z + z^2/2 to <= 3e-4 relative, and the whole
    tail fuses into ONE custom DVE pass:
        out = x * (c1_j + sb_j * (z + 0.5*z^2)),   c1_j = 1 + sb_j
    The omega clamp is host-verified to never bind (cheap upper bound).

Layout: everything feature-major. The HOST ships x^T as bf16 [d, shard]
(features on partitions) so per-partition DMA lines are large and
contiguous; the device writes the f32 output feature-major as well and
the host transposes it back. Per-feature constants (sb, c1) become
per-partition DVE scalars. Device pipeline per column-chunk:
    DVE or Pool : ax = |x| (DVE bit op) or x^2 (Pool tensor_tensor;
                  Ln(x^2) = 2 Ln|x|, the 1/2 folds into that chunk's weights)
    ACT         : lg = Ln(ax)
    PE          : ps = wmI^T.lg      (accumulating matmuls per 512 cols)
    DVE         : oT = x * (c1 + sb*(ps + ps^2/2))   (one fused custom op)
Input DMAs issue from sync + gpsimd in parallel, stores from sync in chunk
order; a few dummy matmuls on the consts tile warm the PE out of its low
p-state while the input streams in.
With the reference G == 0: sb == 0, c1 == 1 exactly, so out == bf16(x) and
the only error vs the f32 reference is the bf16 rounding of x (<= 2^-8).
"""

import sys

for _p in ("/opt/trn_rl_repo",):
    if _p not in sys.path:
        sys.path.insert(0, _p)

import numpy as np
import ml_dtypes

DIM = 128
N_TOTAL = 32768
N_CORES = 8
SHARD = N_TOTAL // N_CORES          # 4096 rows per core
EPS = 1e-07
OMEGA = 20.0

BF16 = ml_dtypes.bfloat16

# column-chunks of the [DIM, SHARD] feature-major tile: small first chunk to
# prime the pipe, small last chunk to shorten the store tail. Each chunk is
# shipped as its OWN contiguous HBM tensor: a [128, sz] slice of the shared
# [128, 4096] tensor reads 2KB segments strided 8KB apart (~190 GB/s
# observed); a contiguous per-chunk tensor is one linear read (~330 GB/s).
_CHUNKS = [(0, 512), (512, 1024), (1536, 1024), (2560, 1024), (3584, 512)]
# Ln-input engine per chunk: DVE x^2 (bf16 tensor_tensor at 2 elem/cycle;
# Ln(x^2) = 2*Ln|x|, the 1/2 folds into the weights), one chunk on the
# otherwise-idle Pool to offload DVE
_AX_ENG = ["dve", "dve", "pool", "dve", "dve"]

# consts tile columns: wmI | wmI/2 | -L/2 | sb(f32 2 cols) | c1(f32) | pad
_C_WMI = 0
_C_WMI2 = DIM
_C_MLH = 2 * DIM
_C_SB = 3 * DIM
_C_C1 = 3 * DIM + 2
_C_COLS = 3 * DIM + 8

_N_WARMUP = 4                       # dummy 256-col matmuls before real work

_PROGRAMS = {}
_DVE_OP = None


def _patch_act_tables(bacc_mod):
    """Make Ln/Exp resolve only to the combined natural_log_exp set, so the
    table-load pass emits a single ACT_TABLE_LOAD for the Ln chain."""
    from concourse import mybir

    orig = bacc_mod.get_activation_tables
    if getattr(orig, "_nalu_patched", False):
        return

    def patched(module_arch):
        tabs = orig(module_arch)
        both = {mybir.ActivationFunctionType.Ln, mybir.ActivationFunctionType.Exp}
        for name, fns in tabs.items():
            if name != "natural_log_exp_and_others":
                fns -= both
        return tabs

    patched._nalu_patched = True
    bacc_mod.get_activation_tables = patched


def _get_dve_op():
    """Register (once) the fused NALU tail as a custom DVE op:
        out = Src1 * (C1 + C0 * (Src0 + Src0^2 * imm2))
    with Src0 = eps_mm (psum f32), Src1 = x (bf16), C0 = sb[j], C1 = c1[j]
    per-partition f32 scalars, imm2 = 0.5."""
    global _DVE_OP
    if _DVE_OP is not None:
        return _DVE_OP
    from concourse import dve_ops
    from concourse.dve_spec import Spec, Src0, Src1, C0, C1, C2, sq, lower

    name = "NALU_V_FUSED_ANT"
    for op in dve_ops.OPS:
        if op.name == name:
            _DVE_OP = op
            return op
    spec = Spec(body=Src1 * (C1 + C0 * (Src0 + sq(Src0) * C2)))
    row = max(dve_ops._SUB_OPCODE_FOR_NAME.values()) + 1
    dve_ops._SUB_OPCODE_FOR_NAME[name] = row
    shas = {}
    for ver in ("v3", "v4"):
        shas[ver] = dve_ops.DveOpSpec(
            name=name, opcode=row, uops=lower(spec, ver=ver),
            rd1_en=dve_ops.has_src1(spec),
        ).sha(ver)
    op = dve_ops.DveOp(name, spec, subdim=False, uops_sha=shas)
    dve_ops.OPS.append(op)
    dve_ops.CUSTOM_DVE_SPECS[name] = spec
    _DVE_OP = op
    return op


def _build_program(use_sg):
    from concourse import bacc, mybir
    from concourse.tile import TileContext

    _patch_act_tables(bacc)
    dve_op = _get_dve_op()

    f32 = mybir.dt.float32
    bf16 = mybir.dt.bfloat16
    u16 = mybir.dt.uint16
    Alu = mybir.AluOpType
    Act = mybir.ActivationFunctionType

    nc = bacc.Bacc("TRN2", target_bir_lowering=False)

    xt_ins = [
        nc.declare_dram_parameter(f"xt{c}", [DIM, sz], bf16, isOutput=False)
        for c, (beg, sz) in enumerate(_CHUNKS)
    ]
    c_in = nc.declare_dram_parameter("consts", [DIM, _C_COLS], bf16, isOutput=False)
    out_ext = nc.declare_dram_parameter("out", [DIM, SHARD], f32, isOutput=True)

    with TileContext(nc) as tc:
        with (
            tc.tile_pool(name="io", bufs=1) as iopool,
            tc.tile_pool(name="mid", bufs=1) as midpool,
            tc.tile_pool(name="mm_ps", bufs=3, space="PSUM") as mmpool,
            tc.tile_pool(name="wu_ps", bufs=1, space="PSUM") as wupool,
        ):
            # consts issue from gpsimd (a DMA on the scalar queue would
            # trigger an extra ACT table load there); the first x chunk owns
            # the sync queue and the DMA bus immediately
            ct = iopool.tile([DIM, _C_COLS], bf16, tag="consts")
            nc.gpsimd.dma_start(ct[:, :], c_in[:, :])
            wmi_t = ct[:, _C_WMI : _C_WMI + DIM]
            wmi2_t = ct[:, _C_WMI2 : _C_WMI2 + DIM]
            mlh_t = ct[:, _C_MLH : _C_MLH + DIM]
            sb_t = ct[:, _C_SB : _C_SB + 2].bitcast(f32)
            c1_t = ct[:, _C_C1 : _C_C1 + 2].bitcast(f32)

            # input chunks all issue from sync in chunk order: serialized
            # issues stagger the transfers so chunk 0 owns the DMA bus first
            xT = iopool.tile([DIM, SHARD], bf16, tag="xT")
            for c, (beg, sz) in enumerate(_CHUNKS):
                cs = slice(beg, beg + sz)
                nc.sync.dma_start(xT[:, cs], xt_ins[c][:, :])

            # PE p-state warmup: stream the consts tile through the array
            wu = wupool.tile([DIM, 256], f32, tag="wu")
            for _ in range(_N_WARMUP):
                nc.tensor.matmul(
                    wu[:], lhsT=wmi_t, rhs=ct[:, 0:256], start=True, stop=True,
                )

            axs = [None] * len(_CHUNKS)
            sgs, lgs, pss = [], [], []

            def emit_ax(c):
                beg, sz = _CHUNKS[c]
                cs = slice(beg, beg + sz)
                ax = midpool.tile([DIM, sz], bf16, tag=f"ax{c}")
                if _AX_ENG[c] == "act":
                    nc.scalar.activation(ax[:], xT[:, cs], Act.Abs)
                elif _AX_ENG[c] == "pool":
                    nc.gpsimd.tensor_tensor(ax[:], xT[:, cs], xT[:, cs], Alu.mult)
                else:
                    nc.vector.tensor_tensor(ax[:], xT[:, cs], xT[:, cs], Alu.mult)
                axs[c] = ax

            def emit_sg(c):
                beg, sz = _CHUNKS[c]
                cs = slice(beg, beg + sz)
                sg = midpool.tile([DIM, sz], bf16, tag=f"sg{c}")
                nc.vector.tensor_scalar(
                    sg[:].bitcast(u16), xT[:, cs].bitcast(u16),
                    0x8000, 0x3F80, Alu.bitwise_and, Alu.bitwise_or,
                )
                sgs[c] = sg

            def emit_ln_mm(c):
                beg, sz = _CHUNKS[c]
                lg = midpool.tile([DIM, sz], bf16, tag=f"lg{c}")
                nc.scalar.activation(lg[:], axs[c][:], Act.Ln)
                # |x| chunks use full-scale weights, x^2 chunks the halved ones
                w_t = wmi_t if _AX_ENG[c] == "act" else wmi2_t
                ps = mmpool.tile([DIM, sz], f32, tag="mm")
                for k in range(sz // 512):
                    ks = slice(k * 512, (k + 1) * 512)
                    nc.tensor.matmul(
                        ps[:, ks], lhsT=w_t, rhs=lg[:, ks],
                        start=True, stop=not use_sg,
                    )
                if use_sg:
                    for k in range(sz // 512):
                        ks = slice(k * 512, (k + 1) * 512)
                        nc.tensor.matmul(
                            ps[:, ks], lhsT=mlh_t, rhs=sgs[c][:, ks],
                            start=False, stop=True,
                        )
                pss[c] = ps

            def emit_fused(c):
                beg, sz = _CHUNKS[c]
                cs = slice(beg, beg + sz)
                # out = x * (c1 + sb*(ps + 0.5*ps^2)) in one fused DVE pass
                oT = midpool.tile([DIM, sz], f32, tag=f"oT{c}")
                nc.vector._custom_dve(
                    dve_op, out=oT[:], in0=pss[c][:], in1=xT[:, cs],
                    s0=sb_t, s1=c1_t, imm2=0.5,
                )
                nc.sync.dma_start(out_ext[:, cs], oT[:])

            sgs = [None] * len(_CHUNKS)
            pss = [None] * len(_CHUNKS)
            for c in range(len(_CHUNKS)):
                emit_ax(c)
                if use_sg:
                    emit_sg(c)
            for c in range(len(_CHUNKS)):
                emit_ln_mm(c)
            for c in range(len(_CHUNKS)):
                emit_fused(c)

    nc.finalize()
    return nc


def _get_program(use_sg=False):
    if use_sg not in _PROGRAMS:
        _PROGRAMS[use_sg] = _build_program(use_sg)
    return _PROGRAMS[use_sg]


def _host_inputs(x, W_m, M_m, G):
    """Host-side parameter precompute shared by kernel() and test harness.

    Returns (in_maps, aux); aux["mode"] is "fast" (fluctuation dropped),
    "sg" (exact sign matmul), or "host" (full CPU fallback)."""
    dim = DIM
    eye = np.eye(dim, dtype=np.float32)
    wm = eye + (1.0 - eye) * np.tanh(W_m) * (1.0 / (1.0 + np.exp(-M_m)))
    wm = wm.astype(np.float32)
    a = np.abs(wm)
    one_m_2a = 1.0 - 2.0 * a
    with np.errstate(divide="ignore"):
        L = np.log(np.abs(one_m_2a)).astype(np.float32)
    np.fill_diagonal(L, 0.0)
    g = np.tanh(G).astype(np.float32)

    # --- device-path validity checks (cheap, O(N d + d^2)) ---------------
    off = one_m_2a.copy()
    np.fill_diagonal(off, 1.0)
    sign_ok = bool((off > 0.0).all())

    xbf = x.astype(BF16)
    absx = np.abs(xbf.astype(np.float32))
    eps_ok = bool((absx >= EPS).all())

    max_absx = float(absx.max()) if absx.size else 1.0
    max_lg = np.log(max(max_absx, EPS))
    maxabs_lg = max(abs(np.log(EPS)), abs(max_lg))
    a_off = a - np.diag(np.diag(a))
    s_off = float(a_off.sum(axis=0).max())
    omega_ok = bool(max_lg + maxabs_lg * s_off < OMEGA - 0.25)
    # Taylor validity: |exp argument| bound small enough for 2nd order
    fl_bound = 0.5 * float(np.abs(L).sum(axis=0).max())
    taylor_ok = bool(maxabs_lg * s_off + fl_bound < 0.25)
    # sign-fluctuation term droppable when its relative effect is tiny
    drop_ok = bool(np.expm1(fl_bound) < 5e-3)

    if sign_ok and eps_ok and omega_ok and taylor_ok:
        mode = "fast" if drop_ok else "sg"
    else:
        mode = "host"

    # --- packed constants -------------------------------------------------
    wmi = (wm - eye).astype(BF16)
    wmi2 = (0.5 * (wm - eye)).astype(BF16)
    mlh = (-0.5 * L).astype(BF16)
    colsum = 0.5 * L.sum(axis=0, dtype=np.float64)
    sb = (g.astype(np.float64) * np.exp(colsum)).astype(np.float32)
    c1 = (1.0 + sb).astype(np.float32)
    sb_u16 = sb.view(np.uint16).reshape(dim, 2)
    c1_u16 = c1.view(np.uint16).reshape(dim, 2)

    consts = np.zeros((dim, _C_COLS), dtype=np.uint16)
    consts[:, _C_WMI : _C_WMI + dim] = wmi.view(np.uint16)
    consts[:, _C_WMI2 : _C_WMI2 + dim] = wmi2.view(np.uint16)
    consts[:, _C_MLH : _C_MLH + dim] = mlh.view(np.uint16)
    consts[:, _C_SB] = sb_u16[:, 0]
    consts[:, _C_SB + 1] = sb_u16[:, 1]
    consts[:, _C_C1] = c1_u16[:, 0]
    consts[:, _C_C1 + 1] = c1_u16[:, 1]
    consts_bf = consts.view(BF16)

    in_maps = []
    for cid in range(N_CORES):
        rows = slice(cid * SHARD, (cid + 1) * SHARD)
        xt = xbf[rows].T
        m = {
            f"xt{c}": np.ascontiguousarray(xt[:, beg : beg + sz])
            for c, (beg, sz) in enumerate(_CHUNKS)
        }
        m["consts"] = consts_bf
        in_maps.append(m)

    aux = {"wm": wm, "a": a, "one_m_2a": one_m_2a, "g": g, "mode": mode}
    return in_maps, aux


def kernel(x, W_m, M_m, G):
    from concourse.bass_utils import run_bass_kernel_spmd

    x = np.asarray(x, dtype=np.float32)
    W_m = np.asarray(W_m, dtype=np.float32)
    M_m = np.asarray(M_m, dtype=np.float32)
    G = np.asarray(G, dtype=np.float32)

    in_maps, aux = _host_inputs(x, W_m, M_m, G)

    if aux["mode"] == "host":
        # General-case fixup (never taken for the reference data):
        # compute the output exactly on the host.
        wm, a, one_m_2a, g = aux["wm"], aux["a"], aux["one_m_2a"], aux["g"]
        lg_h = np.log(np.maximum(np.abs(x), EPS))
        ls = lg_h @ wm
        mul = np.exp(np.minimum(ls, OMEGA))
        msv = np.ones_like(x)
        for i in range(DIM):
            f = np.where(
                x[:, i : i + 1] > 0,
                1.0,
                np.where(x[:, i : i + 1] < 0, one_m_2a[i], 1.0 - a[i]),
            )
            msv *= f
        return (x + mul * msv * g).astype(np.float32)

    nc = _get_program(use_sg=(aux["mode"] == "sg"))
    res = run_bass_kernel_spmd(nc, in_maps, core_ids=list(range(N_CORES)))
    out = np.empty((N_TOTAL, DIM), dtype=np.float32)
    for cid, r in enumerate(res.results):
        rows = slice(cid * SHARD, (cid + 1) * SHARD)
        out[rows] = r["out"].T
    return out


# revision 37
# speedup vs baseline: 1.2129x; 1.0778x over previous
"""NALU layer kernel for Trainium2, data-parallel across 8 NeuronCores.

Reference computation (dim=128, N=32768, eps=1e-7, omega=20):
    wm  = I + (1-I) * tanh(W_m) * sigmoid(M_m)             [d, d]
    ls  = log(max(|x|, eps)) @ wm                          [N, d]
    mul = exp(min(ls, omega))
    msm = sign(x)[:, :, None] * |wm| + (1 - |wm|)          [N, d, d]
    msv = prod(msm, axis=1)                                [N, d]
    out = x + mul * msv * tanh(G)

Restructure (no [N,d,d] product, no on-device transposes, x factored out,
exp replaced by a 2nd-order Taylor of its provably-tiny argument):
    With sigma = sign(x) in {-1,+1} (x==0 / |x|<eps host-checked), and
    L[i,j] = log|1-2|wm[i,j]||  (L[j,j]=0 since |wm[j,j]|=1),
        msv[n,j] = sigma[n,j] * exp( 0.5*colsum_L[j] - sigma[n,:] @ (L[:,j]/2) )
    (off-diagonal (1-2|wm|) > 0 host-verified; diagonal carries the sign).
    Since exp(lg[n,j]) = |x[n,j]| (no |x|<eps, host-verified):
        out[n,j] = x * (1 + sb_j * exp(eps_mm[n,j] + fl[n,j]))
        eps_mm   = lg @ (wm - I)
        fl       = -sigma @ (L/2)        (zero-mean sign fluctuation)
        sb_j     = tanh(G_j) * exp(0.5*colsum_L[j])   (exactly 0 when G==0)
    |fl| <= 0.5*max_colsum|L| (~3e-3 for these weights): when the
    host-computed bound keeps its effect under 0.5% relative it is dropped
    (comparable to the bf16 input rounding of 0.4%); otherwise an alternate
    program that computes it exactly (one more matmul accumulating
    sigma @ (-L/2)) is used.
    |eps_mm| <= max|lg| * max_colsum_offdiag|wm| (~0.05, host-verified
    < 0.25) so exp(z) = 1 + z + z^2/2 to <= 3e-4 relative, and the whole
    tail fuses into ONE custom DVE pass:
        out = x * (c1_j + sb_j * (z + 0.5*z^2)),   c1_j = 1 + sb_j
    The omega clamp is host-verified to never bind (cheap upper bound).

Layout: everything feature-major. The HOST ships x^T as bf16 [d, shard]
(features on partitions) so per-partition DMA lines are large and
contiguous; the device writes the f32 output feature-major as well and
the host transposes it back. Per-feature constants (sb, c1) become
per-partition DVE scalars. Device pipeline per column-chunk:
    DVE or Pool : ax = |x| (DVE bit op) or x^2 (Pool tensor_tensor;
                  Ln(x^2) = 2 Ln|x|, the 1/2 folds into that chunk's weights)
    ACT         : lg = Ln(ax)
    PE          : ps = wmI^T.lg      (accumulating matmuls per 512 cols)
    DVE         : oT = x * (c1 + sb*(ps + ps^2/2))   (one fused custom op)
Input DMAs issue from sync + gpsimd in parallel, stores from sync in chunk
order; a few dummy matmuls on the consts tile warm the PE out of its low
p-state while the input streams in.
With the reference G == 0: sb == 0, c1 == 1 exactly, so out == bf16(x) and
the only error vs the f32 reference is the bf16 rounding of x (<= 2^-8).
"""

import sys

for _p in ("/opt/trn_rl_repo",):
    if _p not in sys.path:
        sys.path.insert(0, _p)

import numpy as np
import ml_dtypes

DIM = 128
N_TOTAL = 32768
N_CORES = 8
SHARD = N_TOTAL // N_CORES          # 4096 rows per core
EPS = 1e-07
OMEGA = 20.0

BF16 = ml_dtypes.bfloat16

# column-chunks of the [DIM, SHARD] feature-major tile: small first chunk to
# prime the pipe, small last chunk to shorten the store tail
_CHUNKS = [(0, 512), (512, 1024), (1536, 1024), (2560, 1024), (3584, 512)]
# every chunk's Ln input is x^2 on DVE (bf16 tensor_tensor at 2 elem/cycle;
# Ln(x^2) = 2*Ln|x|, the 1/2 folds into the matmul weights). ACT-Abs and
# Pool-x^2 offloads measured neutral-to-worse within run noise.
_AX_ENG = ["dve", "dve", "dve", "dve", "dve"]

# consts tile columns: wmI | wmI/2 | -L/2 | sb(f32 2 cols) | c1(f32) | pad
_C_WMI = 0
_C_WMI2 = DIM
_C_MLH = 2 * DIM
_C_SB = 3 * DIM
_C_C1 = 3 * DIM + 2
_C_COLS = 3 * DIM + 8

_N_WARMUP = 4                       # dummy 256-col matmuls before real work

_PROGRAMS = {}
_DVE_OP = None


def _patch_act_tables(bacc_mod):
    """Make Ln/Exp resolve only to the combined natural_log_exp set, so the
    table-load pass emits a single ACT_TABLE_LOAD for the Ln chain."""
    from concourse import mybir

    orig = bacc_mod.get_activation_tables
    if getattr(orig, "_nalu_patched", False):
        return

    def patched(module_arch):
        tabs = orig(module_arch)
        both = {mybir.ActivationFunctionType.Ln, mybir.ActivationFunctionType.Exp}
        for name, fns in tabs.items():
            if name != "natural_log_exp_and_others":
                fns -= both
        return tabs

    patched._nalu_patched = True
    bacc_mod.get_activation_tables = patched


def _get_dve_op():
    """Register (once) the fused NALU tail as a custom DVE op:
        out = Src1 * (C1 + C0 * (Src0 + Src0^2 * imm2))
    with Src0 = eps_mm (psum f32), Src1 = x (bf16), C0 = sb[j], C1 = c1[j]
    per-partition f32 scalars, imm2 = 0.5."""
    global _DVE_OP
    if _DVE_OP is not None:
        return _DVE_OP
    from concourse import dve_ops
    from concourse.dve_spec import Spec, Src0, Src1, C0, C1, C2, sq, lower

    name = "NALU_V_FUSED_ANT"
    for op in dve_ops.OPS:
        if op.name == name:
            _DVE_OP = op
            return op
    spec = Spec(body=Src1 * (C1 + C0 * (Src0 + sq(Src0) * C2)))
    row = max(dve_ops._SUB_OPCODE_FOR_NAME.values()) + 1
    dve_ops._SUB_OPCODE_FOR_NAME[name] = row
    shas = {}
    for ver in ("v3", "v4"):
        shas[ver] = dve_ops.DveOpSpec(
            name=name, opcode=row, uops=lower(spec, ver=ver),
            rd1_en=dve_ops.has_src1(spec),
        ).sha(ver)
    op = dve_ops.DveOp(name, spec, subdim=False, uops_sha=shas)
    dve_ops.OPS.append(op)
    dve_ops.CUSTOM_DVE_SPECS[name] = spec
    _DVE_OP = op
    return op


def _build_program(use_sg):
    from concourse import bacc, mybir
    from concourse.tile import TileContext

    _patch_act_tables(bacc)
    dve_op = _get_dve_op()

    f32 = mybir.dt.float32
    bf16 = mybir.dt.bfloat16
    u16 = mybir.dt.uint16
    Alu = mybir.AluOpType
    Act = mybir.ActivationFunctionType

    nc = bacc.Bacc("TRN2", target_bir_lowering=False)

    xt_in = nc.declare_dram_parameter("xt", [DIM, SHARD], bf16, isOutput=False)
    c_in = nc.declare_dram_parameter("consts", [DIM, _C_COLS], bf16, isOutput=False)
    out_ext = nc.declare_dram_parameter("out", [DIM, SHARD], f32, isOutput=True)

    with TileContext(nc) as tc:
        with (
            tc.tile_pool(name="io", bufs=1) as iopool,
            tc.tile_pool(name="mid", bufs=1) as midpool,
            tc.tile_pool(name="mm_ps", bufs=3, space="PSUM") as mmpool,
            tc.tile_pool(name="wu_ps", bufs=1, space="PSUM") as wupool,
        ):
            # consts issue from gpsimd (a DMA on the scalar queue would
            # trigger an extra ACT table load there); the first x chunk owns
            # the sync queue and the DMA bus immediately
            ct = iopool.tile([DIM, _C_COLS], bf16, tag="consts")
            nc.gpsimd.dma_start(ct[:, :], c_in[:, :])
            wmi_t = ct[:, _C_WMI : _C_WMI + DIM]
            wmi2_t = ct[:, _C_WMI2 : _C_WMI2 + DIM]
            mlh_t = ct[:, _C_MLH : _C_MLH + DIM]
            sb_t = ct[:, _C_SB : _C_SB + 2].bitcast(f32)
            c1_t = ct[:, _C_C1 : _C_C1 + 2].bitcast(f32)

            # input chunks all issue from sync in chunk order: serialized
            # issues stagger the transfers so chunk 0 owns the DMA bus first
            # (parallel issues from several engines made every chunk land
            # together, late; per-chunk contiguous source tensors measured
            # no faster — the ~1.5us/256KB spacing is a bus/HBM property)
            xT = iopool.tile([DIM, SHARD], bf16, tag="xT")
            for c, (beg, sz) in enumerate(_CHUNKS):
                cs = slice(beg, beg + sz)
                nc.sync.dma_start(xT[:, cs], xt_in[:, cs])

            # PE p-state warmup: stream the consts tile through the array
            wu = wupool.tile([DIM, 256], f32, tag="wu")
            for _ in range(_N_WARMUP):
                nc.tensor.matmul(
                    wu[:], lhsT=wmi_t, rhs=ct[:, 0:256], start=True, stop=True,
                )

            axs = [None] * len(_CHUNKS)
            sgs, lgs, pss = [], [], []

            def emit_ax(c):
                beg, sz = _CHUNKS[c]
                cs = slice(beg, beg + sz)
                ax = midpool.tile([DIM, sz], bf16, tag=f"ax{c}")
                if _AX_ENG[c] == "act":
                    nc.scalar.activation(ax[:], xT[:, cs], Act.Abs)
                elif _AX_ENG[c] == "pool":
                    nc.gpsimd.tensor_tensor(ax[:], xT[:, cs], xT[:, cs], Alu.mult)
                else:
                    nc.vector.tensor_tensor(ax[:], xT[:, cs], xT[:, cs], Alu.mult)
                axs[c] = ax

            def emit_sg(c):
                beg, sz = _CHUNKS[c]
                cs = slice(beg, beg + sz)
                sg = midpool.tile([DIM, sz], bf16, tag=f"sg{c}")
                nc.vector.tensor_scalar(
                    sg[:].bitcast(u16), xT[:, cs].bitcast(u16),
                    0x8000, 0x3F80, Alu.bitwise_and, Alu.bitwise_or,
                )
                sgs[c] = sg

            def emit_ln_mm(c):
                beg, sz = _CHUNKS[c]
                lg = midpool.tile([DIM, sz], bf16, tag=f"lg{c}")
                nc.scalar.activation(lg[:], axs[c][:], Act.Ln)
                # |x| chunks use full-scale weights, x^2 chunks the halved ones
                w_t = wmi_t if _AX_ENG[c] == "act" else wmi2_t
                ps = mmpool.tile([DIM, sz], f32, tag="mm")
                for k in range(sz // 512):
                    ks = slice(k * 512, (k + 1) * 512)
                    nc.tensor.matmul(
                        ps[:, ks], lhsT=w_t, rhs=lg[:, ks],
                        start=True, stop=not use_sg,
                    )
                if use_sg:
                    for k in range(sz // 512):
                        ks = slice(k * 512, (k + 1) * 512)
                        nc.tensor.matmul(
                            ps[:, ks], lhsT=mlh_t, rhs=sgs[c][:, ks],
                            start=False, stop=True,
                        )
                pss[c] = ps

            def emit_fused(c):
                beg, sz = _CHUNKS[c]
                cs = slice(beg, beg + sz)
                # out = x * (c1 + sb*(ps + 0.5*ps^2)) in one fused DVE pass
                oT = midpool.tile([DIM, sz], f32, tag=f"oT{c}")
                nc.vector._custom_dve(
                    dve_op, out=oT[:], in0=pss[c][:], in1=xT[:, cs],
                    s0=sb_t, s1=c1_t, imm2=0.5,
                )
                nc.sync.dma_start(out_ext[:, cs], oT[:])

            sgs = [None] * len(_CHUNKS)
            pss = [None] * len(_CHUNKS)
            for c in range(len(_CHUNKS)):
                emit_ax(c)
                if use_sg:
                    emit_sg(c)
            for c in range(len(_CHUNKS)):
                emit_ln_mm(c)
            for c in range(len(_CHUNKS)):
                emit_fused(c)

    nc.finalize()
    return nc


def _get_program(use_sg=False):
    if use_sg not in _PROGRAMS:
        _PROGRAMS[use_sg] = _build_program(use_sg)
    return _PROGRAMS[use_sg]


def _host_inputs(x, W_m, M_m, G):
    """Host-side parameter precompute shared by kernel() and test harness.

    Returns (in_maps, aux); aux["mode"] is "fast" (fluctuation dropped),
    "sg" (exact sign matmul), or "host" (full CPU fallback)."""
    dim = DIM
    eye = np.eye(dim, dtype=np.float32)
    wm = eye + (1.0 - eye) * np.tanh(W_m) * (1.0 / (1.0 + np.exp(-M_m)))
    wm = wm.astype(np.float32)
    a = np.abs(wm)
    one_m_2a = 1.0 - 2.0 * a
    with np.errstate(divide="ignore"):
        L = np.log(np.abs(one_m_2a)).astype(np.float32)
    np.fill_diagonal(L, 0.0)
    g = np.tanh(G).astype(np.float32)

    # --- device-path validity checks (cheap, O(N d + d^2)) ---------------
    off = one_m_2a.copy()
    np.fill_diagonal(off, 1.0)
    sign_ok = bool((off > 0.0).all())

    xbf = x.astype(BF16)
    absx = np.abs(xbf.astype(np.float32))
    eps_ok = bool((absx >= EPS).all())

    max_absx = float(absx.max()) if absx.size else 1.0
    max_lg = np.log(max(max_absx, EPS))
    maxabs_lg = max(abs(np.log(EPS)), abs(max_lg))
    a_off = a - np.diag(np.diag(a))
    s_off = float(a_off.sum(axis=0).max())
    omega_ok = bool(max_lg + maxabs_lg * s_off < OMEGA - 0.25)
    # Taylor validity: |exp argument| bound small enough for 2nd order
    fl_bound = 0.5 * float(np.abs(L).sum(axis=0).max())
    taylor_ok = bool(maxabs_lg * s_off + fl_bound < 0.25)
    # sign-fluctuation term droppable when its relative effect is tiny
    drop_ok = bool(np.expm1(fl_bound) < 5e-3)

    if sign_ok and eps_ok and omega_ok and taylor_ok:
        mode = "fast" if drop_ok else "sg"
    else:
        mode = "host"

    # --- packed constants -------------------------------------------------
    wmi = (wm - eye).astype(BF16)
    wmi2 = (0.5 * (wm - eye)).astype(BF16)
    mlh = (-0.5 * L).astype(BF16)
    colsum = 0.5 * L.sum(axis=0, dtype=np.float64)
    sb = (g.astype(np.float64) * np.exp(colsum)).astype(np.float32)
    c1 = (1.0 + sb).astype(np.float32)
    sb_u16 = sb.view(np.uint16).reshape(dim, 2)
    c1_u16 = c1.view(np.uint16).reshape(dim, 2)

    consts = np.zeros((dim, _C_COLS), dtype=np.uint16)
    consts[:, _C_WMI : _C_WMI + dim] = wmi.view(np.uint16)
    consts[:, _C_WMI2 : _C_WMI2 + dim] = wmi2.view(np.uint16)
    consts[:, _C_MLH : _C_MLH + dim] = mlh.view(np.uint16)
    consts[:, _C_SB] = sb_u16[:, 0]
    consts[:, _C_SB + 1] = sb_u16[:, 1]
    consts[:, _C_C1] = c1_u16[:, 0]
    consts[:, _C_C1 + 1] = c1_u16[:, 1]
    consts_bf = consts.view(BF16)

    in_maps = []
    for cid in range(N_CORES):
        rows = slice(cid * SHARD, (cid + 1) * SHARD)
        in_maps.append(
            {
                "xt": np.ascontiguousarray(xbf[rows].T),
                "consts": consts_bf,
            }
        )

    aux = {"wm": wm, "a": a, "one_m_2a": one_m_2a, "g": g, "mode": mode}
    return in_maps, aux


def kernel(x, W_m, M_m, G):
    from concourse.bass_utils import run_bass_kernel_spmd

    x = np.asarray(x, dtype=np.float32)
    W_m = np.asarray(W_m, dtype=np.float32)
    M_m = np.asarray(M_m, dtype=np.float32)
    G = np.asarray(G, dtype=np.float32)

    in_maps, aux = _host_inputs(x, W_m, M_m, G)

    if aux["mode"] == "host":
        # General-case fixup (never taken for the reference data):
        # compute the output exactly on the host.
        wm, a, one_m_2a, g = aux["wm"], aux["a"], aux["one_m_2a"], aux["g"]
        lg_h = np.log(np.maximum(np.abs(x), EPS))
        ls = lg_h @ wm
        mul = np.exp(np.minimum(ls, OMEGA))
        msv = np.ones_like(x)
        for i in range(DIM):
            f = np.where(
                x[:, i : i + 1] > 0,
                1.0,
                np.where(x[:, i : i + 1] < 0, one_m_2a[i], 1.0 - a[i]),
            )
            msv *= f
        return (x + mul * msv * g).astype(np.float32)

    nc = _get_program(use_sg=(aux["mode"] == "sg"))
    res = run_bass_kernel_spmd(nc, in_maps, core_ids=list(range(N_CORES)))
    out = np.empty((N_TOTAL, DIM), dtype=np.float32)
    for cid, r in enumerate(res.results):
        rows = slice(cid * SHARD, (cid + 1) * SHARD)
        out[rows] = r["out"].T
    return out
